# revision 23
# baseline (speedup 1.0000x reference)
"""Trainium2 Bass kernel: causal linear attention MHA (nn_MHA_72413148610860).

Math (per (batch, head), D=64, S=2048):
    q,k,v = x @ w + b          (H=512 -> D=64 per head)
    pq,pk = elu(q)+1, elu(k)+1
    out_s = (pq_s . sum_{t<=s} pk_t v_t^T) / (pq_s . sum_{t<=s} pk_t + eps)
    y = out @ wo + bo

Sharding: 8 cores = 2 batches x 4 head-pairs. Each core computes a partial
output projection over its 2 heads; host sums the 4 partials per batch.

Device algorithm: chunked linear attention, chunk C=128, 16 chunks. Matmul
operands are bf16 (fp32 PSUM accumulation, fp32 recurrent state + output).
"""

import sys
import numpy as np

sys.path.insert(0, "/opt/trn_rl_repo")

B, S, H = 2, 2048, 512
N, D = 8, 64
EPS = 1e-6
C = 128          # chunk length
NCHUNK = S // C  # 16
ND = 128         # 2 heads * D on device
KC = H // 128    # 4 contraction sub-tiles

_CACHE = {}


def _build_nc():
    import concourse.bass as bass
    import concourse.mybir as mybir
    from concourse import bacc, tile

    f32 = mybir.dt.float32
    bf16 = mybir.dt.bfloat16
    A = mybir.AluOpType
    Act = mybir.ActivationFunctionType

    nc = bacc.Bacc("TRN2", target_bir_lowering=False, debug=False, num_devices=8)

    def din(name, shape, dt=bf16):
        return nc.dram_tensor(name, shape, dt, kind="ExternalInput")

    xq = din("xq", [H, S])
    xk = din("xk", [H, S])
    xv = din("xv", [H, S])
    wq = din("wq", [H, ND])
    wk = din("wk", [H, ND])
    wv = din("wv", [H, ND])
    bq = din("bq", [ND, 1], f32)
    bk = din("bk", [ND, 1], f32)
    bvc = din("bvc", [ND, 1], f32)
    wo = din("wo", [ND, H])
    umask = din("umask", [C, C], f32)
    bdmask = din("bdmask", [ND, 130], f32)
    ident = din("ident", [128, 128])
    mh0 = din("mh0", [ND, 1], f32)
    mh1 = din("mh1", [ND, 1], f32)
    out = nc.dram_tensor("out", [S, H], f32, kind="ExternalOutput")

    with tile.TileContext(nc) as tc:
        with (
            tc.tile_pool(name="const", bufs=1) as cp,
            tc.tile_pool(name="xin", bufs=3) as xp,
            tc.tile_pool(name="work", bufs=3) as wp,
            tc.tile_pool(name="outs", bufs=16) as op,
            tc.tile_pool(name="pproj", bufs=3, space="PSUM") as pp,
            tc.tile_pool(name="pat", bufs=2, space="PSUM") as pa,
            tc.tile_pool(name="pnum", bufs=2, space="PSUM") as pw,
            tc.tile_pool(name="pout", bufs=1, space="PSUM") as po,
        ):
            # ---- constants / weights (resident), staged through DVE so PE
            # only ever waits on one semaphore (HW LDW struct wait limit) ----
            def staged(name, shape, dram, dt=bf16):
                tmp = cp.tile(shape, dt, tag=name + "_t")
                dst = cp.tile(shape, dt, tag=name)
                nc.sync.dma_start(out=tmp[:], in_=dram[:])
                nc.vector.tensor_copy(dst[:], tmp[:])
                return dst

            def _stage_w(name, w_dr):
                wtmp = cp.tile([128, H], bf16, tag=name + "_t")
                w_sb = cp.tile([128, H], bf16, tag=name)
                nc.sync.dma_start(
                    out=wtmp.rearrange("p (kc n) -> p kc n", kc=KC),
                    in_=w_dr.rearrange("(kc p) n -> p kc n", p=128),
                )
                nc.vector.tensor_copy(w_sb[:], wtmp[:])
                return w_sb

            wq_sb = _stage_w("wq", wq)
            wk_sb = _stage_w("wk", wk)
            wv_sb = _stage_w("wv", wv)
            wo_sb = staged("wo", [128, H], wo)
            bq_sb = staged("bq", [128, 1], bq, f32)
            bk_sb = staged("bk", [128, 1], bk, f32)
            bv_sb = staged("bv", [128, 1], bvc, f32)
            um_sb = staged("um", [C, C], umask, f32)
            bd_sb = staged("bd", [ND, 130], bdmask, f32)
            id_sb = staged("id", [128, 128], ident)
            mh0_sb = staged("mh0", [128, 1], mh0, f32)
            mh1_sb = staged("mh1", [128, 1], mh1, f32)

            # persistent recurrent state: block-diag S (64x64 per head) plus
            # z column per head (col h*65+64). fp32 accumulator + bf16 shadow
            st_sb = cp.tile([ND, 130], f32, tag="state")
            nc.vector.memset(st_sb[:], 0.0)
            st_bf = cp.tile([ND, 130], bf16, tag="state_bf")
            nc.vector.memset(st_bf[:], 0.0)

            # sacrificial PE op: PE observes the setup DVE tick
            dummy = po.tile([128, 128], bf16, tag="outp")
            nc.tensor.matmul(dummy[:], id_sb[:], id_sb[:], is_transpose=True,
                             start=True, stop=True, skip_group_check=True)

            x_tiles = [None, None, None]
            for ci in range(NCHUNK):
                s0 = ci * C
                # ---- load x (host-transposed [H, S]) every 2 chunks ----
                if ci % 2 == 0:
                    x_tiles = [
                        xp.tile([128, 2 * H], bf16, name="xq_t", tag="xq"),
                        xp.tile([128, 2 * H], bf16, name="xk_t", tag="xk"),
                        xp.tile([128, 2 * H], bf16, name="xv_t", tag="xv"),
                    ]
                    for t_sb, x_dr in zip(x_tiles, (xq, xk, xv)):
                        nc.sync.dma_start(
                            out=t_sb.rearrange("p (kc s) -> p kc s", kc=KC),
                            in_=x_dr.rearrange("(kc p) s -> p kc s", p=128)[
                                :, :, s0 : s0 + 2 * C
                            ],
                        )
                xq_t, xk_t, xv_t = x_tiles
                e = (ci % 2) * C

                def xsl(kc):
                    return slice(kc * 2 * C + e, kc * 2 * C + e + C)

                # ---- projections: qT,kT -> [nd, s]; v -> [s, nd] ----
                qt_ps = pp.tile([128, C], f32, tag="proj")
                kt_ps = pp.tile([128, C], f32, tag="proj")
                v_ps = pp.tile([128, C], f32, tag="proj")
                for dst, lhs, rhs in ((qt_ps, wq_sb, xq_t), (kt_ps, wk_sb, xk_t)):
                    for kc in range(KC):
                        nc.tensor.matmul(
                            dst[:], lhs[:, kc * 128 : (kc + 1) * 128],
                            rhs[:, xsl(kc)],
                            start=(kc == 0), stop=(kc == KC - 1),
                        )
                for kc in range(KC):
                    nc.tensor.matmul(
                        v_ps[:], xv_t[:, xsl(kc)],
                        wv_sb[:, kc * 128 : (kc + 1) * 128],
                        start=(kc == 0), stop=(kc == KC - 1),
                    )

                # ---- feature map phi(x) = relu(x+b) + exp(min(x+b, 0)) ----
                pq_sb = wp.tile([128, C], bf16, tag="pq")
                pk_sb = wp.tile([128, C], bf16, tag="pk")
                for (dst, src, b_sb) in (
                    (pq_sb, qt_ps[:], bq_sb),
                    (pk_sb, kt_ps[:], bk_sb),
                ):
                    mn = wp.tile([128, C], f32, tag="mn")
                    ex = wp.tile([128, C], f32, tag="ex")
                    rl = wp.tile([128, C], f32, tag="rl")
                    nc.vector.tensor_scalar(mn[:], src, b_sb[:], 0.0, A.add, A.min)
                    nc.scalar.activation(ex[:], mn[:], Act.Exp)
                    nc.scalar.activation(rl[:], src, Act.Relu, bias=b_sb[:])
                    nc.vector.tensor_add(dst[:], ex[:], rl[:])

                # masked per-head copies of pk for the AT matmuls
                pkz0 = wp.tile([128, C], bf16, tag="pkz0")
                pkz1 = wp.tile([128, C], bf16, tag="pkz1")
                nc.vector.tensor_scalar_mul(pkz0[:], pk_sb[:], mh0_sb[:])
                nc.vector.tensor_scalar_mul(pkz1[:], pk_sb[:], mh1_sb[:])

                # pk transposed back to [s, nd] (lhsT of the state delta)
                pkT_ps = pa.tile([128, C], bf16, tag="at")
                nc.tensor.transpose(pkT_ps[:], pk_sb[:], id_sb[:])
                pkT_sb = wp.tile([128, C], bf16, tag="pkT")
                nc.scalar.activation(pkT_sb[:], pkT_ps[:], Act.Copy)

                # ---- AT_h[s_k, s_q] = sum_d pk_h[d,s_k] pq[d,s_q] ----
                at0_ps = pa.tile([C, C], f32, tag="at")
                at1_ps = pa.tile([C, C], f32, tag="at")
                nc.tensor.matmul(at0_ps[:], pkz0[:], pq_sb[:], start=True, stop=True)
                nc.tensor.matmul(at1_ps[:], pkz1[:], pq_sb[:], start=True, stop=True)
                atm0 = wp.tile([C, C], bf16, tag="atm0")
                atm1 = wp.tile([C, C], bf16, tag="atm1")
                nc.vector.tensor_mul(atm0[:], at0_ps[:], um_sb[:])
                nc.vector.tensor_mul(atm1[:], at1_ps[:], um_sb[:])

                # ---- per-head zero-padded v_ext [s, 130]:
                #      cols h*65..h*65+63 = v_h, col h*65+64 = ones ----
                vx = []
                for h in range(2):
                    vh = wp.tile([C, 130], bf16, tag=f"vext{h}")
                    o = (1 - h) * 65
                    nc.vector.memset(vh[:, o : o + 65], 0.0)
                    nc.vector.tensor_copy(
                        vh[:, h * 65 : h * 65 + 64],
                        v_ps[:, h * 64 : (h + 1) * 64],
                    )
                    nc.vector.memset(vh[:, h * 65 + 64 : h * 65 + 65], 1.0)
                    vx.append(vh)

                # ---- numerator+denominator [s, 130]: one 3-matmul group ----
                num_ps = pw.tile([C, 130], f32, tag="num")
                nc.tensor.matmul(num_ps[:], atm0[:], vx[0][:], start=True, stop=False)
                nc.tensor.matmul(num_ps[:], atm1[:], vx[1][:], start=False, stop=False)
                nc.tensor.matmul(num_ps[:], pq_sb[:], st_bf[:], start=False, stop=True)

                # ---- state delta [nd, 130] and masked accumulate ----
                del_ps = pw.tile([ND, 130], f32, tag="num")
                nc.tensor.matmul(del_ps[:], pkT_sb[:], vx[0][:], start=True, stop=False)
                nc.tensor.matmul(del_ps[:], pkT_sb[:], vx[1][:], start=False, stop=True)
                stmp = wp.tile([ND, 130], f32, tag="stmp")
                nc.vector.tensor_mul(stmp[:], del_ps[:], bd_sb[:])
                nc.vector.tensor_add(st_sb[:], st_sb[:], stmp[:])
                nc.vector.tensor_copy(st_bf[:], st_sb[:])

                # ---- divide ----
                dtmp = wp.tile([C, 2], f32, tag="dtmp")
                rec = wp.tile([C, 2], f32, tag="rec")
                nc.vector.tensor_scalar_add(
                    dtmp.rearrange("p (h x) -> p h x", x=1),
                    num_ps[:].rearrange("p (h x) -> p h x", x=65)[:, :, 64:65],
                    EPS,
                )
                nc.vector.reciprocal(rec[:], dtmp[:])
                av_sb = wp.tile([C, ND], bf16, tag="av")
                for h in range(2):
                    nc.vector.tensor_scalar_mul(
                        av_sb[:, h * 64 : (h + 1) * 64],
                        num_ps[:, h * 65 : h * 65 + 64],
                        rec[:, h : h + 1],
                    )

                # ---- output projection ----
                avT_ps = pa.tile([ND, C], bf16, tag="at")
                nc.tensor.transpose(avT_ps[:], av_sb[:], id_sb[:])
                avT_sb = wp.tile([ND, C], bf16, tag="avT")
                nc.vector.tensor_scalar_add(avT_sb[:], avT_ps[:], bv_sb[:])
                outp = po.tile([C, H], f32, tag="outp")
                nc.tensor.matmul(outp[:], avT_sb[:], wo_sb[:],
                                 start=True, stop=True)
                out_sb = op.tile([C, H], f32, tag="out")
                nc.vector.tensor_copy(out_sb[:], outp[:])
                nc.sync.dma_start(out=out[s0 : s0 + C, :], in_=out_sb[:])

    nc.compile()
    return nc


def _get_nc():
    if "nc" not in _CACHE:
        _CACHE["nc"] = _build_nc()
    return _CACHE["nc"]


def _host_inputs(query, key, value, wq, bq, wk, bk, wv, bv, wo):
    """Build the 8 per-core input maps."""
    import ml_dtypes

    bf = ml_dtypes.bfloat16
    umask = np.triu(np.ones((C, C), np.float32))
    bdmask = np.zeros((ND, 130), np.float32)
    for h in range(2):
        bdmask[h * 64 : (h + 1) * 64, h * 65 : (h + 1) * 65] = 1.0
    ident = np.eye(128, dtype=bf)
    mh0 = np.zeros((ND, 1), np.float32)
    mh0[:64] = 1.0
    mh1 = np.zeros((ND, 1), np.float32)
    mh1[64:] = 1.0

    xT = {}
    for b in range(B):
        xT[("q", b)] = np.ascontiguousarray(query[b].T).astype(bf)
        xT[("k", b)] = np.ascontiguousarray(key[b].T).astype(bf)
        xT[("v", b)] = np.ascontiguousarray(value[b].T).astype(bf)

    in_maps = []
    for c in range(8):
        b, p = c // 4, c % 4
        hsl = slice(2 * p, 2 * p + 2)
        in_maps.append(
            {
                "xq": xT[("q", b)],
                "xk": xT[("k", b)],
                "xv": xT[("v", b)],
                "wq": np.ascontiguousarray(wq[:, hsl, :].reshape(H, ND)).astype(bf),
                "wk": np.ascontiguousarray(wk[:, hsl, :].reshape(H, ND)).astype(bf),
                "wv": np.ascontiguousarray(wv[:, hsl, :].reshape(H, ND)).astype(bf),
                "bq": np.ascontiguousarray(bq[hsl].reshape(ND, 1)),
                "bk": np.ascontiguousarray(bk[hsl].reshape(ND, 1)),
                "bvc": np.ascontiguousarray(bv[hsl].reshape(ND, 1)),
                "wo": np.ascontiguousarray(wo[hsl].reshape(ND, H)).astype(bf),
                "umask": umask,
                "bdmask": bdmask,
                "ident": ident,
                "mh0": mh0,
                "mh1": mh1,
            }
        )
    return in_maps


def kernel(query, key, value, wq, bq, wk, bk, wv, bv, wo, bo, step, **_):
    from concourse.bass_utils import run_bass_kernel_spmd

    query = np.asarray(query, np.float32)
    key = np.asarray(key, np.float32)
    value = np.asarray(value, np.float32)
    in_maps = _host_inputs(
        query, key, value,
        np.asarray(wq, np.float32), np.asarray(bq, np.float32),
        np.asarray(wk, np.float32), np.asarray(bk, np.float32),
        np.asarray(wv, np.float32), np.asarray(bv, np.float32),
        np.asarray(wo, np.float32),
    )
    nc = _get_nc()
    res = run_bass_kernel_spmd(nc, in_maps, list(range(8)))
    out = np.zeros((B, S, H), np.float32)
    for c in range(8):
        out[c // 4] += res.results[c]["out"]
    out += np.asarray(bo, np.float32)
    return out
